# revision 11
# baseline (speedup 1.0000x reference)
"""Masked phase-locking value (PLV) kernel for Trainium2, 8 NeuronCores.

Math: out[b] = |sum_ij M_ij * exp(i*(a_bi - b_bj))| / max(sum(M), 1)

Device decomposition (per core, Na sharded 8 ways -> 1024 i-columns each):
    Z[c, i] = sum_j cs2[j, c] * M[i, j]        (TensorE, fp8 DoubleRow)
with cs2 = [cb; sb] stacked along c (c = 2B = 128) as the STATIONARY
operand and the transposed mask streaming through. The whole Nb=8192
contraction accumulates in PSUM (one bank per 512-wide i-block), so the
epilogue shrinks to the sharded dim and fuses into one DVE op per side:
    racc[c] = sum_i Z[c, i] * WA[c, i]    (scalar_tensor_tensor accum_out)
    qacc[c] = sum_i Z[c, i] * WS[c, i]
with WA = [ca; sa], WS = [sa; -ca] (sign folded in on host), giving
real_b = sum_cores racc[b] + racc[64+b], imag_b likewise from qacc.

dtypes: mask 0/1 and cs2 in fp8e4 -> DoubleRow double-pumped matmul
(2 contraction rows/cycle); WA/WS fp16; PSUM/epilogue fp32. Host-emulated
end-to-end rel err 2.3e-3 (gate 2e-2).

The kernel is HBM-DMA-bound (~9.5 MB/core; all 8 cores stream at once so
the early phase is chip-contention limited). Mask DMAs are paced by a
4-deep tile pool (~3-4 transfers in flight - queuing everything up front
spreads SDMA completions and stalls the PE). Column groups are sized
small-big-small per i-block: a small first group starts the PE early,
a small last group plus the fused one-op epilogue keeps the tail short.
cs2 rides the scalar HWDGE ring in 8-chunk pieces paced ahead of use; a
PE warm-up burst during the DMA lead-in defeats the HAM cold-clock
penalty.
"""

import numpy as np

import concourse.bass as bass
import concourse.tile as tile
from concourse import bacc, mybir
from concourse.bass_utils import run_bass_kernel_spmd

B = 64
NA = 8192
NB = 8192
NCORES = 8
NASH = NA // NCORES          # i-columns per core
JC = NB // 256               # 32 DoubleRow j-chunks of 256

NIB = 2                      # i-blocks of 512 (one PSUM bank each)
IBW = NASH // NIB
# j-chunks per mask DMA group, per i-block
GJC = [[4, 8, 12, 8], [8, 12, 8, 4]]
assert all(sum(g) == JC for g in GJC)

NWU = 18                     # PE warm-up matmuls during DMA lead-in

F8 = mybir.dt.float8e4
F16 = mybir.dt.float16
F32 = mybir.dt.float32
DR = mybir.MatmulPerfMode.DoubleRow
BYP = mybir.AluOpType.bypass
MUL = mybir.AluOpType.mult


def build_program() -> bass.Bass:
    nc = bacc.Bacc("TRN2")
    # mask: concatenated group blocks, each contiguous [128, gjc, 2, 512]
    mask_d = nc.dram_tensor("mask", [128 * JC * 2 * NASH], F8, kind="ExternalInput")
    # cs2: 4 contiguous pieces of [128, 8, 2, 128]
    cs2_d = nc.dram_tensor("cs2", [128 * JC * 2 * 128], F8, kind="ExternalInput")
    wv_d = nc.dram_tensor("wv", [128, 2, NASH], F16, kind="ExternalInput")
    out_d = nc.dram_tensor("out", [128, 2 * NIB], F32, kind="ExternalOutput")

    with tile.TileContext(nc) as tc:
        with (
            tc.tile_pool(name="consts", bufs=1) as consts,
            tc.tile_pool(name="masks", bufs=4) as masks,
            tc.tile_pool(name="junk", bufs=2) as junkp,
            tc.tile_pool(name="zpsum", bufs=1, space="PSUM") as zpool,
            tc.tile_pool(name="wups", bufs=1, space="PSUM") as wu_pool,
        ):
            # scalar HWDGE ring: 4 cs2 pieces (paced ahead of use), then wv
            cs2_sb = consts.tile([128, JC, 2, 128], F8)
            psz = 128 * 8 * 2 * 128
            for piece in range(4):
                src = cs2_d[piece * psz : (piece + 1) * psz].rearrange(
                    "(p j t c) -> p j t c", p=128, j=8, t=2
                )
                nc.scalar.dma_start(out=cs2_sb[:, 8 * piece : 8 * piece + 8], in_=src)
            wv_sb = consts.tile([128, 2, NASH], F16)
            nc.scalar.dma_start(out=wv_sb[:], in_=wv_d[:])

            # PE warm-up on a memset tile while the first mask group is in
            # flight (HAM clock ramp)
            wu8 = consts.tile([128, 128], F8)
            nc.vector.memset(wu8[:], 1.0)
            wu_ps = wu_pool.tile([128, 128], F32)
            for r in range(NWU):
                nc.tensor.matmul(
                    out=wu_ps[:], lhsT=wu8[:], rhs=wu8[:],
                    start=(r == 0), stop=(r == NWU - 1),
                )

            racc = consts.tile([128, 2 * NIB], F32)
            off = 0
            for ib in range(NIB):
                z = zpool.tile([128, IBW], F32, tag=f"z{ib}")
                jc = 0
                for gjc in GJC[ib]:
                    sz = 128 * gjc * 2 * IBW
                    mt = masks.tile([128, gjc, 2, IBW], F8, tag="mask")
                    src = mask_d[off : off + sz].rearrange(
                        "(p k t i) -> p k t i", p=128, k=gjc, t=2
                    )
                    nc.sync.dma_start(out=mt[:], in_=src)
                    off += sz
                    for k in range(gjc):
                        nc.tensor.matmul(
                            out=z[:],
                            lhsT=cs2_sb[:, jc],
                            rhs=mt[:, k],
                            start=(jc == 0),
                            stop=(jc == JC - 1),
                            perf_mode=DR,
                        )
                        jc += 1
                isl = slice(ib * IBW, (ib + 1) * IBW)
                for q in range(2):  # 0: real-side (WA), 1: imag-side (WS)
                    jr = junkp.tile([128, IBW], F32, tag="junk")
                    nc.vector.scalar_tensor_tensor(
                        out=jr[:], in0=z[:], scalar=0.0, in1=wv_sb[:, q, isl],
                        op0=BYP, op1=MUL,
                        accum_out=racc[:, 2 * ib + q : 2 * ib + q + 1],
                    )
            nc.scalar.dma_start(out=out_d[:], in_=racc[:])
    nc.finalize()
    return nc


def prep_inputs(phases_a, phases_b, coupling_mask):
    pa = np.asarray(phases_a, dtype=np.float32)
    pb = np.asarray(phases_b, dtype=np.float32)
    ca, sa = np.cos(pa), np.sin(pa)
    cb, sb = np.cos(pb), np.sin(pb)
    f8np = mybir.dt.np(F8)

    # cs2 pieces: [p, jc, t, c] with j = jc*256 + t*128 + p, piece-contiguous
    CS2 = np.concatenate([cb, sb], axis=0).astype(f8np)     # [c, j]
    csv = CS2.T.reshape(4, 8, 2, 128, 128)                  # [piece, jck, t, p, c]
    cs2_host = np.ascontiguousarray(csv.transpose(0, 3, 1, 2, 4)).reshape(-1)

    one_byte = np.array([1.0], f8np).view(np.uint8)[0]
    mask_u8 = (np.asarray(coupling_mask) != 0).astype(np.uint8) * one_byte
    MT = np.ascontiguousarray(mask_u8.T)                    # [j, i]

    WA = np.concatenate([ca, sa], axis=0)                   # [c, i_full]
    WS = np.concatenate([sa, -ca], axis=0)

    in_maps = []
    for c in range(NCORES):
        isl = slice(c * NASH, (c + 1) * NASH)
        # [jc, t, p, ib, ii] -> [ib, jc, p, t, ii]
        v = MT[:, isl].reshape(JC, 2, 128, NIB, IBW).transpose(3, 0, 2, 1, 4)
        blocks = []
        for ib in range(NIB):
            jc = 0
            for gjc in GJC[ib]:
                blk = v[ib, jc : jc + gjc].transpose(1, 0, 2, 3)  # [p, k, t, ii]
                blocks.append(np.ascontiguousarray(blk).reshape(-1))
                jc += gjc
        m_host = np.concatenate(blocks).view(f8np)
        wv = np.stack([WA[:, isl], WS[:, isl]], axis=1).astype(np.float16)
        in_maps.append({"mask": m_host, "cs2": cs2_host, "wv": wv})
    return in_maps


def combine(outs, coupling_mask):
    o = np.stack(outs).astype(np.float64)   # [NCORES, 128, 2*NIB]
    r = o[:, :, 0::2].sum(axis=2)           # [NCORES, 128]
    q = o[:, :, 1::2].sum(axis=2)
    real = (r[:, :B] + r[:, B:]).sum(axis=0)
    imag = (q[:, :B] + q[:, B:]).sum(axis=0)
    n_pairs = max(float(np.asarray(coupling_mask).sum()), 1.0)
    return (np.sqrt(real * real + imag * imag) / n_pairs).astype(np.float32)


_prog_cache: list = []


def kernel(phases_a, phases_b, coupling_mask):
    in_maps = prep_inputs(phases_a, phases_b, coupling_mask)
    if not _prog_cache:
        _prog_cache.append(build_program())
    res = run_bass_kernel_spmd(_prog_cache[0], in_maps, core_ids=list(range(NCORES)))
    return combine([r["out"] for r in res.results], coupling_mask)


# revision 13
# speedup vs baseline: 1.0269x; 1.0269x over previous
"""Masked phase-locking value (PLV) kernel for Trainium2, 8 NeuronCores.

Math: out[b] = |sum_ij M_ij * exp(i*(a_bi - b_bj))| / max(sum(M), 1)

Device decomposition (per core, Na sharded 8 ways -> 1024 i-columns each):
    Z[c, i] = sum_j cs2[j, c] * M[i, j]        (TensorE, fp8 DoubleRow)
with cs2 = [cb; sb] stacked along c (c = 2B = 128) as the STATIONARY
operand and the transposed mask streaming through. The whole Nb=8192
contraction accumulates in PSUM (one bank per 512-wide i-block), so the
epilogue shrinks to the sharded dim and fuses into one DVE op per side:
    racc[c] = sum_i Z[c, i] * WA[c, i]    (scalar_tensor_tensor accum_out)
    qacc[c] = sum_i Z[c, i] * WS[c, i]
with WA = [ca; sa], WS = [sa; -ca] (sign folded in on host), giving
real_b = sum_cores racc[b] + racc[64+b], imag_b likewise from qacc.

dtypes: mask 0/1 and cs2 in fp8e4 -> DoubleRow double-pumped matmul
(2 contraction rows/cycle); WA/WS fp16; PSUM/epilogue fp32. Host-emulated
end-to-end rel err 2.3e-3 (gate 2e-2).

The kernel is HBM-DMA-bound (~9.5 MB/core; all 8 cores stream at once so
the early phase is chip-contention limited). Mask DMAs are paced by a
4-deep tile pool (~3-4 transfers in flight - queuing everything up front
spreads SDMA completions and stalls the PE). Column groups are sized
small-big-small per i-block: a small first group starts the PE early,
a small last group plus the fused one-op epilogue keeps the tail short.
cs2 rides the scalar HWDGE ring in 8-chunk pieces paced ahead of use; a
PE warm-up burst during the DMA lead-in defeats the HAM cold-clock
penalty.
"""

import numpy as np

import concourse.bass as bass
import concourse.tile as tile
from concourse import bacc, mybir
from concourse.bass_utils import run_bass_kernel_spmd

B = 64
NA = 8192
NB = 8192
NCORES = 8
NASH = NA // NCORES          # i-columns per core
JC = NB // 256               # 32 DoubleRow j-chunks of 256

NIB = 2                      # i-blocks of 512 (one PSUM bank each)
IBW = NASH // NIB
# j-chunks per mask DMA group, per i-block
GJC = [[4, 8, 12, 8], [8, 12, 8, 4]]
assert all(sum(g) == JC for g in GJC)

NWU = 9                      # PE warm-up matmuls (N=512: ~3.8us continuous
                             # busy - enough to flip the HAM clock to 2.4GHz)

F8 = mybir.dt.float8e4
F16 = mybir.dt.float16
F32 = mybir.dt.float32
DR = mybir.MatmulPerfMode.DoubleRow
BYP = mybir.AluOpType.bypass
MUL = mybir.AluOpType.mult


def build_program() -> bass.Bass:
    nc = bacc.Bacc("TRN2")
    # mask: concatenated group blocks, each contiguous [128, gjc, 2, 512]
    mask_d = nc.dram_tensor("mask", [128 * JC * 2 * NASH], F8, kind="ExternalInput")
    # cs2: 4 contiguous pieces of [128, 8, 2, 128]
    cs2_d = nc.dram_tensor("cs2", [128 * JC * 2 * 128], F8, kind="ExternalInput")
    wv_d = nc.dram_tensor("wv", [128, 2, NASH], F16, kind="ExternalInput")
    out_d = nc.dram_tensor("out", [128, 2 * NIB], F32, kind="ExternalOutput")

    with tile.TileContext(nc) as tc:
        with (
            tc.tile_pool(name="consts", bufs=1) as consts,
            tc.tile_pool(name="masks", bufs=4) as masks,
            tc.tile_pool(name="junk", bufs=2) as junkp,
            tc.tile_pool(name="zpsum", bufs=1, space="PSUM") as zpool,
            tc.tile_pool(name="wups", bufs=1, space="PSUM") as wu_pool,
        ):
            # scalar HWDGE ring: 4 cs2 pieces (paced ahead of use), then wv
            cs2_sb = consts.tile([128, JC, 2, 128], F8)
            psz = 128 * 8 * 2 * 128
            for piece in range(4):
                src = cs2_d[piece * psz : (piece + 1) * psz].rearrange(
                    "(p j t c) -> p j t c", p=128, j=8, t=2
                )
                nc.scalar.dma_start(out=cs2_sb[:, 8 * piece : 8 * piece + 8], in_=src)
            wv_sb = consts.tile([128, 2, NASH], F16)
            nc.scalar.dma_start(out=wv_sb[:], in_=wv_d[:])

            # PE warm-up on a memset tile while the first mask group is in
            # flight (HAM clock ramp)
            wu8 = consts.tile([128, 128], F8)
            nc.vector.memset(wu8[:], 1.0)
            wuR = consts.tile([128, 512], F8)
            nc.vector.memset(wuR[:], 1.0)
            wu_ps = wu_pool.tile([128, 512], F32)
            for r in range(NWU):
                nc.tensor.matmul(
                    out=wu_ps[:], lhsT=wu8[:], rhs=wuR[:],
                    start=(r == 0), stop=(r == NWU - 1),
                )

            racc = consts.tile([128, 2 * NIB], F32)
            off = 0
            for ib in range(NIB):
                z = zpool.tile([128, IBW], F32, tag=f"z{ib}")
                jc = 0
                for gjc in GJC[ib]:
                    sz = 128 * gjc * 2 * IBW
                    mt = masks.tile([128, gjc, 2, IBW], F8, tag="mask")
                    src = mask_d[off : off + sz].rearrange(
                        "(p k t i) -> p k t i", p=128, k=gjc, t=2
                    )
                    nc.sync.dma_start(out=mt[:], in_=src)
                    off += sz
                    for k in range(gjc):
                        nc.tensor.matmul(
                            out=z[:],
                            lhsT=cs2_sb[:, jc],
                            rhs=mt[:, k],
                            start=(jc == 0),
                            stop=(jc == JC - 1),
                            perf_mode=DR,
                        )
                        jc += 1
                isl = slice(ib * IBW, (ib + 1) * IBW)
                for q in range(2):  # 0: real-side (WA), 1: imag-side (WS)
                    jr = junkp.tile([128, IBW], F32, tag="junk")
                    nc.vector.scalar_tensor_tensor(
                        out=jr[:], in0=z[:], scalar=0.0, in1=wv_sb[:, q, isl],
                        op0=BYP, op1=MUL,
                        accum_out=racc[:, 2 * ib + q : 2 * ib + q + 1],
                    )
            nc.scalar.dma_start(out=out_d[:], in_=racc[:])
    nc.finalize()
    return nc


def prep_inputs(phases_a, phases_b, coupling_mask):
    pa = np.asarray(phases_a, dtype=np.float32)
    pb = np.asarray(phases_b, dtype=np.float32)
    ca, sa = np.cos(pa), np.sin(pa)
    cb, sb = np.cos(pb), np.sin(pb)
    f8np = mybir.dt.np(F8)

    # cs2 pieces: [p, jc, t, c] with j = jc*256 + t*128 + p, piece-contiguous
    CS2 = np.concatenate([cb, sb], axis=0).astype(f8np)     # [c, j]
    csv = CS2.T.reshape(4, 8, 2, 128, 128)                  # [piece, jck, t, p, c]
    cs2_host = np.ascontiguousarray(csv.transpose(0, 3, 1, 2, 4)).reshape(-1)

    one_byte = np.array([1.0], f8np).view(np.uint8)[0]
    mask_u8 = (np.asarray(coupling_mask) != 0).astype(np.uint8) * one_byte
    MT = np.ascontiguousarray(mask_u8.T)                    # [j, i]

    WA = np.concatenate([ca, sa], axis=0)                   # [c, i_full]
    WS = np.concatenate([sa, -ca], axis=0)

    in_maps = []
    for c in range(NCORES):
        isl = slice(c * NASH, (c + 1) * NASH)
        # [jc, t, p, ib, ii] -> [ib, jc, p, t, ii]
        v = MT[:, isl].reshape(JC, 2, 128, NIB, IBW).transpose(3, 0, 2, 1, 4)
        blocks = []
        for ib in range(NIB):
            jc = 0
            for gjc in GJC[ib]:
                blk = v[ib, jc : jc + gjc].transpose(1, 0, 2, 3)  # [p, k, t, ii]
                blocks.append(np.ascontiguousarray(blk).reshape(-1))
                jc += gjc
        m_host = np.concatenate(blocks).view(f8np)
        wv = np.stack([WA[:, isl], WS[:, isl]], axis=1).astype(np.float16)
        in_maps.append({"mask": m_host, "cs2": cs2_host, "wv": wv})
    return in_maps


def combine(outs, coupling_mask):
    o = np.stack(outs).astype(np.float64)   # [NCORES, 128, 2*NIB]
    r = o[:, :, 0::2].sum(axis=2)           # [NCORES, 128]
    q = o[:, :, 1::2].sum(axis=2)
    real = (r[:, :B] + r[:, B:]).sum(axis=0)
    imag = (q[:, :B] + q[:, B:]).sum(axis=0)
    n_pairs = max(float(np.asarray(coupling_mask).sum()), 1.0)
    return (np.sqrt(real * real + imag * imag) / n_pairs).astype(np.float32)


_prog_cache: list = []


def kernel(phases_a, phases_b, coupling_mask):
    in_maps = prep_inputs(phases_a, phases_b, coupling_mask)
    if not _prog_cache:
        _prog_cache.append(build_program())
    res = run_bass_kernel_spmd(_prog_cache[0], in_maps, core_ids=list(range(NCORES)))
    return combine([r["out"] for r in res.results], coupling_mask)
